# revision 4
# baseline (speedup 1.0000x reference)
"""MoE routing kernel for Trainium2 (8 NeuronCores, batch-parallel).

Problem: nn_MoE_47278999994656.
  x [8, 256, 80, 80] f32 + gate Linear(256->5) + 5 experts
  (residual conv1x1 on each 128-ch half, gated by a sigmoid transform),
  top-1 masked-softmax gate => weights are EXACTLY one-hot, so
  out[b] = expert_{argmax_e logits[b,e]}(x[b]).

Sharding: data-parallel over batch, core i computes batch item i.

Restructured dataflow (vs the straightforward D->H->A->combine):
  reference: D_h = (I+W_h) x_h + b_h ; s_h = sigmoid(Wt2 relu(Wt1 D_h + bt1) + bt2)
             out = s0*D0 + s1*D1   (s per-column scalars)
  Because per-column scaling commutes with channel mixing, and with
  gamma_h = (I+W_h)^{-1} b_h (host-precomputed, well-conditioned):
     s_h * D_h = (I+W_h) (s_h * (x_h + gamma_h))
  and the sigmoid path folds to  h = [Wt1(I+W_h)] x_h + (Wt1 b_h + bt1),
  so the kernel per 512-col chunk is:
     H-matmuls (from x) -> relu -> A-matmuls (replicated Wt2, psum pair)
     -> ONE paired sigmoid -> y_h = (x_h + gamma_h)*s_h (fused DVE stt)
     -> final matmuls (I+W_h)^T accumulated -> evict -> DMA out.
  This deletes the D evictions and the 3-op combine entirely.

x is host-cast to bf16 ([128, 2, HW] layout), output returned bf16 and
host-upcast. PE is kept continuously busy (junk matmuls) through phase 1
and the select window so phase-2 matmuls run at the full 2.4 GHz p-state.
Phase 2 is software-pipelined: PE iteration i issues H(i), A(i-1), F(i-2)
so the PE never waits on the Act/DVE post-processing chain.
"""

import numpy as np

import concourse.bacc as bacc_mod
import concourse.bass as bass
import concourse.mybir as mybir
import concourse.tile as tile
from concourse.bass_utils import run_bass_kernel_spmd

B, C, H, W = 8, 256, 80, 80
HW = H * W          # 6400
HALF = 128
QUARTER = 64
E = 5
NCORES = 8

# phase-2 chunks: 12 x 512 + 1 x 256 (psum bank holds 512 f32)
CHUNKS = [(i * 512, 512) for i in range(12)] + [(6144, 256)]
NCH = len(CHUNKS)

# input x DMA chunks (columns); 8 x 800
XCH = [(i * 800, 800) for i in range(8)]

# U free-dim layout (per expert, partition dim = 128):
#   [0:128)    F_rgb = (I + Wrgb)^T          [c, o]
#   [128:256)  F_tir = (I + Wtir)^T          [c, o]
#   [256:320)  Wh_rgb^T = ((Wt1 @ F_rgb^T))^T [c, m]  (m = 64)
#   [320:384)  Wh_tir^T                       [c, m]
#   [384]      wt2 stacked: row r = Wt2[r % 64]  (expanded to 128 cols on dev)
UF = 385
U_F0 = 0
U_F1 = 128
U_WH0 = 256
U_WH1 = 320
U_WT2 = 384
USEL_F = 512          # selected-weights tile: wt2 expanded to [384:512)

# bias table columns: 0 = relu bias (Wt1 b_h + bt1, halves stacked),
# 1 = bt2 (replicated), 2 = gamma_rgb, 3 = gamma_tir
NBIAS = 4

F32 = mybir.dt.float32
BF16 = mybir.dt.bfloat16


def build_nc() -> bass.Bass:
    nc = bacc_mod.Bacc()

    x_d = nc.dram_tensor("x", [HALF, 2, HW], BF16, kind="ExternalInput")
    u_d = nc.dram_tensor("u", [HALF, E, UF], BF16, kind="ExternalInput")
    bias_d = nc.dram_tensor("bias", [HALF, E, NBIAS], F32, kind="ExternalInput")
    wg_d = nc.dram_tensor("wg", [HALF, 2, E], BF16, kind="ExternalInput")
    bg_d = nc.dram_tensor("bg", [1, E], F32, kind="ExternalInput")
    out_d = nc.dram_tensor("out", [HALF, HW], BF16, kind="ExternalOutput")

    with tile.TileContext(nc) as tc:
        with (
            tc.tile_pool(name="big", bufs=1) as big,
            tc.tile_pool(name="const", bufs=1) as const,
            tc.tile_pool(name="small", bufs=1) as small,
            tc.tile_pool(name="sb", bufs=3) as sb,
            tc.tile_pool(name="ps", bufs=2, space="PSUM") as ps,
        ):
            # ---- persistent SBUF ----
            xb = big.tile([HALF, 2, HW], BF16)       # 25.6 KB/part
            osb = big.tile([HALF, HW], BF16)         # 12.8 KB/part
            u_all = const.tile([HALF, E, UF], BF16)
            bias_all = const.tile([HALF, E, NBIAS], F32)
            wg = const.tile([HALF, 2, E], BF16)
            bgx = const.tile([1, E], F32)
            usel = const.tile([HALF, USEL_F], BF16)
            bsel = const.tile([HALF, NBIAS], F32)
            ones128 = const.tile([HALF, HALF], BF16)
            ones1 = const.tile([1, HALF], F32)
            t32a = small.tile([32, 32], F32)
            t32b = small.tile([32, 32], F32)

            # ---- DMA dispatch: weights on gpsimd ring, x on sync ring ----
            nc.gpsimd.dma_start(out=u_all[:], in_=u_d[:])
            nc.gpsimd.dma_start(out=bias_all[:], in_=bias_d[:])
            nc.gpsimd.dma_start(out=wg[:], in_=wg_d[:])
            nc.gpsimd.dma_start(out=bgx[:], in_=bg_d[:])
            for o, n in XCH:
                nc.sync.dma_start(out=xb[:, :, o : o + n], in_=x_d[:, :, o : o + n])

            nc.vector.memset(t32a, 0.0)
            nc.vector.memset(ones1, 1.0)
            nc.vector.memset(ones128, 1.0)

            # junk matmuls: PE warmth (p-state ramp) through phase 1 + select
            def junk():
                jp = ps.tile([HALF, 512], F32, tag="hps")
                nc.tensor.matmul(
                    jp[:, 0:UF], lhsT=u_all[:, 0, 0:HALF], rhs=u_all[:, 0, :]
                )

            # ---- phase 1: gate logits, overlapped with x DMA ----
            yg = ps.tile([E, 512], F32, tag="outps")
            gsl = []
            for o, n in XCH:
                for h in range(2):
                    gsl.append((h, o, 512))
                    gsl.append((h, o + 512, n - 512))
            junk()
            junk()
            for k, (h, o, n) in enumerate(gsl):
                nc.tensor.matmul(
                    yg[:, 0:n],
                    lhsT=wg[:, h, :],
                    rhs=xb[:, h, o : o + n],
                    start=(k == 0),
                    stop=(k == len(gsl) - 1),
                )
                if k % 2 == 1:
                    junk()

            # gate epilogue (all DVE until the mask broadcast)
            l51 = small.tile([E, 1], F32)
            nc.vector.reduce_sum(l51, yg, axis=mybir.AxisListType.X)
            nc.vector.tensor_copy(t32a[0:E, 0:1], l51)
            junk()
            nc.vector.transpose(t32b, t32a)
            lrow = small.tile([1, E], F32)
            nc.vector.tensor_add(lrow, t32b[0:1, 0:E], bgx[0:1, :])
            lmax = small.tile([1, 1], F32)
            nc.vector.reduce_max(lmax, lrow, axis=mybir.AxisListType.X)
            mrow = small.tile([1, E], F32)
            nc.vector.tensor_scalar(
                out=mrow, in0=lrow, scalar1=lmax, scalar2=None,
                op0=mybir.AluOpType.is_equal,
            )
            junk()
            mps = ps.tile([HALF, 512], F32, tag="outps")
            nc.tensor.matmul(mps[:, 0:E], lhsT=ones1, rhs=mrow)
            mbc = small.tile([HALF, E], F32)
            nc.scalar.activation(
                out=mbc, in_=mps[:, 0:E],
                func=mybir.ActivationFunctionType.Copy,
            )
            junk()

            # ---- select expert weights (mask is exactly one-hot) ----
            # DVE: Wh ranges first (needed by first H matmuls), then wt2,
            # then F ranges (needed 2 iterations later).
            # gpsimd: bias table (needed by first relu).
            def sel_range(lo, hi):
                nc.vector.tensor_scalar(
                    out=usel[:, lo:hi], in0=u_all[:, 0, lo:hi],
                    scalar1=mbc[:, 0:1], scalar2=None,
                    op0=mybir.AluOpType.mult,
                )
                for e in range(1, E):
                    nc.vector.scalar_tensor_tensor(
                        out=usel[:, lo:hi], in0=u_all[:, e, lo:hi],
                        scalar=mbc[:, e : e + 1], in1=usel[:, lo:hi],
                        op0=mybir.AluOpType.mult, op1=mybir.AluOpType.add,
                    )

            sel_range(U_WH0, U_WH1 + QUARTER)      # [256:384)
            junk()
            nc.vector.tensor_scalar(
                out=bsel, in0=bias_all[:, 0, :],
                scalar1=mbc[:, 0:1], scalar2=None,
                op0=mybir.AluOpType.mult,
            )
            for e in range(1, E):
                nc.vector.scalar_tensor_tensor(
                    out=bsel, in0=bias_all[:, e, :],
                    scalar=mbc[:, e : e + 1], in1=bsel,
                    op0=mybir.AluOpType.mult, op1=mybir.AluOpType.add,
                )
            # wt2: select the single column, then expand to 128 cols
            wt2c = small.tile([HALF, 1], F32)
            nc.vector.tensor_scalar(
                out=wt2c, in0=u_all[:, 0, U_WT2 : U_WT2 + 1],
                scalar1=mbc[:, 0:1], scalar2=None, op0=mybir.AluOpType.mult,
            )
            for e in range(1, E):
                nc.vector.scalar_tensor_tensor(
                    out=wt2c, in0=u_all[:, e, U_WT2 : U_WT2 + 1],
                    scalar=mbc[:, e : e + 1], in1=wt2c,
                    op0=mybir.AluOpType.mult, op1=mybir.AluOpType.add,
                )
            nc.vector.tensor_scalar(
                out=usel[:, U_WT2 : U_WT2 + HALF], in0=ones128,
                scalar1=wt2c, scalar2=None, op0=mybir.AluOpType.mult,
            )
            junk()
            sel_range(U_F0, U_F1 + HALF)           # [0:256)
            junk()
            junk()

            # ---- phase 2: software-pipelined H -> relu -> A -> sigmoid
            #      -> y=(x+gamma)*s -> final matmuls -> evict -> DMA out ----
            hsb = [None] * NCH
            ssb = [None] * NCH
            ytl = [None] * NCH
            apsl = [None] * NCH
            opsl = [None] * NCH

            for i in range(NCH + 2):
                if i < NCH:
                    off, n = CHUNKS[i]
                    hps = ps.tile([HALF, 512], F32, tag="hps")
                    nc.tensor.matmul(
                        hps[0:QUARTER, 0:n],
                        lhsT=usel[:, U_WH0 : U_WH0 + QUARTER],
                        rhs=xb[:, 0, off : off + n],
                    )
                    nc.tensor.matmul(
                        hps[QUARTER:HALF, 0:n],
                        lhsT=usel[:, U_WH1 : U_WH1 + QUARTER],
                        rhs=xb[:, 1, off : off + n],
                        tile_position=(0, QUARTER),
                    )
                    hsb[i] = sb.tile([HALF, 512], BF16, tag="hsb", name="hsb")
                    nc.scalar.activation(
                        out=hsb[i][:, 0:n], in_=hps[:, 0:n],
                        func=mybir.ActivationFunctionType.Relu,
                        bias=bsel[:, 0:1],
                    )
                if 0 <= i - 1 < NCH:
                    j = i - 1
                    off, n = CHUNKS[j]
                    apsl[j] = ps.tile([HALF, 2, 512], F32, tag="aps", name="apsl")
                    nc.tensor.matmul(
                        apsl[j][:, 0, 0:n],
                        lhsT=usel[0:QUARTER, U_WT2 : U_WT2 + HALF],
                        rhs=hsb[j][0:QUARTER, 0:n],
                        tile_position=(0, 0),
                    )
                    nc.tensor.matmul(
                        apsl[j][:, 1, 0:n],
                        lhsT=usel[QUARTER:HALF, U_WT2 : U_WT2 + HALF],
                        rhs=hsb[j][QUARTER:HALF, 0:n],
                        tile_position=(QUARTER, 0),
                    )
                    ssb[j] = sb.tile([HALF, 2, 512], BF16, tag="ssb", name="ssb")
                    nc.scalar.activation(
                        out=ssb[j][:, :, 0:n], in_=apsl[j][:, :, 0:n],
                        func=mybir.ActivationFunctionType.Sigmoid,
                        bias=bsel[:, 1:2],
                    )
                    ytl[j] = sb.tile([HALF, 2, 512], BF16, tag="y", name="ytl")
                    for h in range(2):
                        nc.vector.scalar_tensor_tensor(
                            out=ytl[j][:, h, 0:n],
                            in0=xb[:, h, off : off + n],
                            scalar=bsel[:, 2 + h : 3 + h],
                            in1=ssb[j][:, h, 0:n],
                            op0=mybir.AluOpType.add,
                            op1=mybir.AluOpType.mult,
                        )
                if 0 <= i - 2 < NCH:
                    j = i - 2
                    off, n = CHUNKS[j]
                    opsl[j] = ps.tile([HALF, 512], F32, tag="outps", name="opsl")
                    nc.tensor.matmul(
                        opsl[j][:, 0:n], lhsT=usel[:, U_F0 : U_F0 + HALF],
                        rhs=ytl[j][:, 0, 0:n], start=True, stop=False,
                    )
                    nc.tensor.matmul(
                        opsl[j][:, 0:n], lhsT=usel[:, U_F1 : U_F1 + HALF],
                        rhs=ytl[j][:, 1, 0:n], start=False, stop=True,
                    )
                    nc.vector.tensor_copy(
                        osb[:, off : off + n], opsl[j][:, 0:n]
                    )
                    # batched output DMA every 2 chunks
                    if j % 2 == 1:
                        bo = CHUNKS[j - 1][0]
                        bn = off + n - bo
                        nc.sync.dma_start(
                            out=out_d[:, bo : bo + bn], in_=osb[:, bo : bo + bn]
                        )
                    elif j == NCH - 1:
                        nc.sync.dma_start(
                            out=out_d[:, off : off + n],
                            in_=osb[:, off : off + n],
                        )
                if i == 0:
                    junk()
                    junk()
                if i == 1:
                    junk()

    nc.compile()
    return nc


def _pack_inputs(x, Wg, bg, Wrgb, brgb, Wtir, btir, Wt1, bt1, Wt2, bt2):
    import ml_dtypes

    eye = np.eye(HALF, dtype=np.float64)
    u = np.zeros((E, HALF, UF), dtype=np.float64)
    biasT = np.zeros((E, HALF, NBIAS), dtype=np.float64)
    for e in range(E):
        F0 = eye + Wrgb[e].astype(np.float64)
        F1 = eye + Wtir[e].astype(np.float64)
        Wt1e = Wt1[e].astype(np.float64)
        u[e, :, U_F0 : U_F0 + HALF] = F0.T
        u[e, :, U_F1 : U_F1 + HALF] = F1.T
        u[e, :, U_WH0 : U_WH0 + QUARTER] = (Wt1e @ F0).T
        u[e, :, U_WH1 : U_WH1 + QUARTER] = (Wt1e @ F1).T
        u[e, :, U_WT2] = np.tile(Wt2[e, 0].astype(np.float64), 2)
        biasT[e, 0:QUARTER, 0] = Wt1e @ brgb[e].astype(np.float64) + bt1[e]
        biasT[e, QUARTER:HALF, 0] = Wt1e @ btir[e].astype(np.float64) + bt1[e]
        biasT[e, :, 1] = bt2[e, 0]
        biasT[e, :, 2] = np.linalg.solve(F0, brgb[e].astype(np.float64))
        biasT[e, :, 3] = np.linalg.solve(F1, btir[e].astype(np.float64))
    u = np.ascontiguousarray(u.transpose(1, 0, 2)).astype(ml_dtypes.bfloat16)
    bias = np.ascontiguousarray(biasT.transpose(1, 0, 2)).astype(np.float32)

    wgt = Wg.T.astype(np.float32)                   # [256, 5]
    wg_p = np.ascontiguousarray(
        np.stack([wgt[:HALF], wgt[HALF:]], axis=1)
    ).astype(ml_dtypes.bfloat16)                    # [128, 2, 5]
    bgx = np.ascontiguousarray((bg * float(HW))[None, :].astype(np.float32))

    xp = np.ascontiguousarray(
        x.reshape(B, 2, HALF, HW).transpose(0, 2, 1, 3)
    ).astype(ml_dtypes.bfloat16)                    # [B, 128, 2, HW]

    common = {"u": u, "bias": bias, "wg": wg_p, "bg": bgx}
    in_maps = []
    for b in range(B):
        m = dict(common)
        m["x"] = xp[b]
        in_maps.append(m)
    return in_maps


_NC_CACHE = {}


def _get_nc():
    if "nc" not in _NC_CACHE:
        _NC_CACHE["nc"] = build_nc()
    return _NC_CACHE["nc"]


def kernel(x, Wg, bg, Wrgb, brgb, Wtir, btir, Wt1, bt1, Wt2, bt2, **run_kw):
    nc = _get_nc()
    in_maps = _pack_inputs(
        np.asarray(x), np.asarray(Wg), np.asarray(bg), np.asarray(Wrgb),
        np.asarray(brgb), np.asarray(Wtir), np.asarray(btir),
        np.asarray(Wt1), np.asarray(bt1), np.asarray(Wt2), np.asarray(bt2),
    )
    res = run_bass_kernel_spmd(nc, in_maps, core_ids=list(range(NCORES)), **run_kw)
    out = np.stack(
        [np.asarray(r["out"]).astype(np.float32) for r in res.results], axis=0
    )
    if run_kw:
        kernel.last_results = res
    return out.reshape(B, HALF, H, W)
